# revision 23
# baseline (speedup 1.0000x reference)
"""Trainium2 Bass kernel for nn_DiscreteWaveletTransform (3-level db4 DWT,
symmetric padding, + linear resize of each coefficient band back to T).

Approach: the whole per-signal pipeline (3 DWT levels + 4 resizes) is one
fixed linear operator out[t, c] = sum_k sig[k] * M[k, 4t+c].  M (2048 x 8192)
is banded (bandwidth <= 194 rows per 128-wide t-chunk), so each 128-signal
block reduces to 46 PE matmuls of [K=128, M=128 signals, N=512 cols] in
float32r (full-rate fp32 with 11-bit mantissa), accumulated in PSUM over the
2-3 k-blocks that cover each t-chunk's band.  The matmul orientation puts
signals on PSUM partitions and (t, c)-interleaved columns on the free axis,
so each PSUM tile is exactly a contiguous [128 signals, 128 t x 4 c] chunk of
the output layout — no transposes and fully contiguous 2 KiB-per-row output
DMAs.

Sharding: data-parallel over B (16 -> 2 per core); each core handles
2 x 512 = 1024 signals = 8 blocks of 128.
"""

from contextlib import ExitStack

import numpy as np

import concourse.bacc as bacc
import concourse.bass as bass
import concourse.tile as tile
from concourse import mybir
from concourse.bass_utils import run_bass_kernel_spmd

# ---------------------------------------------------------------- problem dims
import os  # noqa: E402

B, T, N = 16, 2048, 512
LEVELS = 3
C = LEVELS + 1
F = 8
NCORES = 8
B_PER_CORE = B // NCORES          # 2
TCHUNKS = T // 128                # 16
SCHUNKS = TCHUNKS + 1             # 17 sample chunks on the 64-shifted grid
SBLOCKS = B_PER_CORE * (N // 128)  # 8 signal blocks of 128 per core
OGROUP = int(os.environ.get("K_OGROUP", "8"))  # t-chunks per output DMA

DEC_LO = np.array([-0.010597401784997278, 0.032883011666982945, 0.030841381835986965,
                   -0.18703481171888114, -0.02798376941698385, 0.6308807679295904,
                   0.7148465705525415, 0.23037781330885523])
DEC_HI = np.array([-0.23037781330885523, 0.7148465705525415, -0.6308807679295904,
                   -0.02798376941698385, 0.18703481171888114, 0.030841381835986965,
                   -0.032883011666982945, -0.010597401784997278])


# ------------------------------------------------------- operator construction
def _dwt_step(sig, lo, hi):
    S = sig.shape[1]
    ext = np.pad(sig, ((0, 0), (F - 1, F - 1)), mode='symmetric')[:, 1:]
    L = (S + F - 1) // 2
    lo_r, hi_r = lo[::-1], hi[::-1]
    cA = sum(ext[:, k:k + 2 * L:2] * lo_r[k] for k in range(F))
    cD = sum(ext[:, k:k + 2 * L:2] * hi_r[k] for k in range(F))
    return cA, cD


def _resize(c, t):
    S = c.shape[-1]
    if S == t:
        return c
    if S > t:
        return c[..., :t]
    pos = (np.arange(t, dtype=c.dtype) + 0.5) * (S / t) - 0.5
    pos = np.clip(pos, 0.0, S - 1)
    lo = np.floor(pos).astype(np.int64)
    hi = np.minimum(lo + 1, S - 1)
    w = pos - lo.astype(c.dtype)
    return c[..., lo] * (1.0 - w) + c[..., hi] * w


def _build_operator():
    """M [T, T, C] float64: out[s, t, c] = sum_k sig[s, k] M[k, t, c]."""
    a = np.eye(T)
    details = []
    for _ in range(LEVELS):
        a, d = _dwt_step(a, DEC_LO, DEC_HI)
        details.append(d)
    coeffs = [a] + details[::-1]
    return np.stack([_resize(cf, T) for cf in coeffs], axis=-1)


def _plan():
    """Banded matmul schedule on the 64-shifted sample grid.

    The signal is staged in SBUF as 17 chunks of 128 samples covering
    [-64, 2112) (64 zero rows at each end).  Every t-chunk's operator band
    (width <= 194) then fits in exactly TWO consecutive shifted chunks
    [128j-64, 128j+192), vs 2-3 chunks (46 total) on the 0-aligned grid.
    Returns per-t-chunk block lists, the (tc, j) pair order, and the packed-M
    array [128, n_pairs, 512] whose partition p holds operator row 128j-64+p.
    """
    M = _build_operator()                      # [k, t, c]
    Mi = M.reshape(T, T * C)                   # col = 4 t + c
    Mpad = np.zeros((T + 128, T * C))          # row s+64 = operator row s
    Mpad[64:64 + T] = Mi
    kblocks, pairs = [], []
    for tc in range(TCHUNKS):
        cols = Mi[:, tc * 512:(tc + 1) * 512]
        rows = np.nonzero(np.any(cols != 0, axis=1))[0]
        jmin = (rows.min() + 64) // 128
        jmax = (rows.max() + 64) // 128
        blocks = list(range(jmin, jmax + 1))
        assert len(blocks) == 2, (tc, blocks)
        kblocks.append(blocks)
        for kb in blocks:
            pairs.append((tc, kb))
    packed = np.empty((128, len(pairs), 512), dtype=np.float32)
    for p, (tc, kb) in enumerate(pairs):
        packed[:, p, :] = Mpad[kb * 128:(kb + 1) * 128, tc * 512:(tc + 1) * 512]
    return kblocks, pairs, packed


_KBLOCKS, _PAIRS, _M_PACKED = _plan()
NPAIRS = len(_PAIRS)

F32 = mybir.dt.float32
F32R = mybir.dt.float32r
BF16 = mybir.dt.bfloat16

# M in bf16 halves the dominant input stream (11.75 -> 5.9 MiB per core).
# Walrus rejects mixed-dtype matmuls, so the signal is cast to bf16 in the
# load DMA (SWDGE casting DMA) when M is bf16.  float32r/float32r is the
# high-accuracy fallback (2.1e-4 vs ~1e-3 relative error).
_DT_CHOICE = os.environ.get("K_DTYPE", "bf16")
M_DTYPE = BF16 if _DT_CHOICE == "bf16" else F32R
SIG_DTYPE = M_DTYPE

if M_DTYPE == BF16:
    import ml_dtypes
    _M_PACKED = _M_PACKED.astype(ml_dtypes.bfloat16)

# Output band: fp16 halves the dominant store stream (32 -> 16 MiB per core)
# at a cost of ~5e-4 output-quantization relative error (fp16 keeps 11
# mantissa bits; |out| <= ~8 is far inside fp16 range).  Host casts back.
F16 = mybir.dt.float16
_ODT_CHOICE = os.environ.get("K_ODT", "f16")
OUT_DTYPE = {"f16": F16, "bf16": BF16, "f32": F32}[_ODT_CHOICE]
OUT_NP = {"f16": np.float16, "bf16": None, "f32": np.float32}[_ODT_CHOICE]


# ------------------------------------------------------------- device program
def _emit_body(tc_ctx, nc, x_d, o_d, pools, m_t, ogroup, probe=None):
    spool, opool, ppool = pools

    pair_idx = {pr: i for i, pr in enumerate(_PAIRS)}
    ncopy = 0
    for b in range(B_PER_CORE):
        for nb in range(N // 128):
            sig = spool.tile([128, SCHUNKS, 128], SIG_DTYPE, name="sig")
            nc.sync.dma_start(sig[:], x_d[b, nb])
            for tc0 in range(0, TCHUNKS, 2):
                # two t-chunks share one 2-bank PSUM tile so the PSUM->SBUF
                # copy moves 1024 elems per instruction (halves per-inst
                # seq/access overhead on the copy engines)
                acc = ppool.tile([128, 2, 512], F32, name="acc")
                for tci in (tc0, tc0 + 1):
                    blocks = _KBLOCKS[tci]
                    nmm = 1 if probe == "dma" else len(blocks)
                    for j, kb in enumerate(blocks[:nmm]):
                        nc.tensor.matmul(
                            acc[:, tci - tc0, :],
                            sig[:, kb, :],
                            m_t[:, pair_idx[(tci, kb)], :],
                            start=(j == 0), stop=(j == nmm - 1),
                        )
                if probe == "pe":
                    continue
                j = tc0 % ogroup
                if j == 0:
                    o_t = opool.tile([128, ogroup, 512], OUT_DTYPE, name="o_t")
                # PSUM -> SBUF (+ f32 -> OUT_DTYPE convert) round-robined over
                # DVE and Activation so neither becomes the serial stage
                if ncopy % 2 == 0:
                    nc.vector.tensor_copy(o_t[:, j:j + 2, :], acc[:])
                else:
                    nc.scalar.copy(o_t[:, j:j + 2, :], acc[:])
                ncopy += 1
                if probe == "nostore":
                    continue
                if j == ogroup - 2:
                    # batched store (OGROUP t-chunks -> one DMA).  Only SP and
                    # Activation have HWDGE rings; split the output stream
                    # between them ~5:3 so ring bytes balance (SP also carries
                    # the input loads: 4.5 MiB in + 3/8 of 16.8 MiB out vs
                    # 5/8 of 16.8 MiB on Activation).
                    t0 = (tc0 - j) * 128
                    eng = nc.sync if (ncopy * 2 // ogroup) % 8 < 3 else nc.scalar
                    eng.dma_start(
                        o_d[b, nb * 128:(nb + 1) * 128, t0:t0 + ogroup * 128, :],
                        o_t[:],
                    )


def build_module(reps=1, ogroup=None, probe=None):
    """Build + compile the per-core Bass module.  reps>1 wraps the body in a
    hardware loop (used by test.py for wall-clock differencing timing)."""
    if ogroup is None:
        ogroup = OGROUP
    nc = bacc.Bacc("TRN2", target_bir_lowering=False, debug=False)
    x_d = nc.dram_tensor("x", [B_PER_CORE, N // 128, 128, SCHUNKS * 128],
                         SIG_DTYPE, kind="ExternalInput")
    m_d = nc.dram_tensor("m", [128, NPAIRS, 512], M_DTYPE, kind="ExternalInput")
    o_d = nc.dram_tensor("out", [B_PER_CORE, N, T, C], OUT_DTYPE,
                         kind="ExternalOutput")

    with tile.TileContext(nc) as tc_ctx, ExitStack() as ctx:
        mpool = ctx.enter_context(tc_ctx.tile_pool(name="mpool", bufs=1))
        pools = (
            ctx.enter_context(tc_ctx.tile_pool(name="spool", bufs=3)),
            ctx.enter_context(tc_ctx.tile_pool(name="opool",
                                             bufs=max(3, 24 // ogroup))),
            ctx.enter_context(tc_ctx.tile_pool(name="ppool", bufs=4, space="PSUM")),
        )
        # M is loop-invariant: one batched load before the rep loop; it stays
        # resident in SBUF, so steady-state iterations stream only x and out.
        m_t = mpool.tile([128, NPAIRS, 512], M_DTYPE, name="m_t")
        nc.sync.dma_start(m_t[:], m_d[:])
        if reps == 1:
            _emit_body(tc_ctx, nc, x_d, o_d, pools, m_t, ogroup, probe)
        else:
            with tc_ctx.For_i(0, reps, 1,
                              hint_engines=(mybir.EngineType.PE,
                                            mybir.EngineType.SP)):
                _emit_body(tc_ctx, nc, x_d, o_d, pools, m_t, ogroup, probe)

    nc.compile()
    return nc


_NC_CACHE = {}


def _get_module(reps=1, ogroup=None, probe=None):
    key = (reps, ogroup, probe)
    if key not in _NC_CACHE:
        _NC_CACHE[key] = build_module(reps, ogroup, probe)
    return _NC_CACHE[key]


# ------------------------------------------------------------------ entrypoint
def run(x, reps=1, ogroup=None, probe=None):
    """x: [16, 2048, 512, 1] float32 -> [16, 512, 2048, 4] float32."""
    nc = _get_module(reps, ogroup, probe)
    x3 = np.asarray(x)[:, :, :, 0]
    if SIG_DTYPE == BF16:
        import ml_dtypes
        x3 = x3.astype(ml_dtypes.bfloat16)
    else:
        x3 = x3.astype(np.float32)
    # pre-tile to the 64-shifted SBUF layout: sample s lives at
    # (chunk (s+64)//128, partition (s+64)%128); 64 zero rows pad each end
    xp = np.zeros((B, SCHUNKS * 128, N), dtype=x3.dtype)
    xp[:, 64:64 + T] = x3
    xt = np.ascontiguousarray(
        xp.reshape(B, SCHUNKS, 128, N // 128, 128).transpose(0, 3, 2, 1, 4)
        .reshape(B, N // 128, 128, SCHUNKS * 128))
    in_maps = [
        {"x": xt[c * B_PER_CORE:(c + 1) * B_PER_CORE], "m": _M_PACKED}
        for c in range(NCORES)
    ]
    res = run_bass_kernel_spmd(nc, in_maps, core_ids=list(range(NCORES)))
    out = np.concatenate([np.asarray(res.results[c]["out"])
                          for c in range(NCORES)], axis=0)
    return np.ascontiguousarray(out.astype(np.float32))


def kernel(x):
    return run(x)



# revision 38
# speedup vs baseline: 1.4965x; 1.4965x over previous
"""Trainium2 Bass kernel for nn_DiscreteWaveletTransform (3-level db4 DWT,
symmetric padding, + linear resize of each coefficient band back to T).

Approach: the whole per-signal pipeline (3 DWT levels + 4 resizes) is one
fixed linear operator out[t, c] = sum_k sig[k] * M[k, 4t+c].  M (2048 x 8192)
is banded (bandwidth <= 194 rows per 128-wide t-chunk), so each 128-signal
block reduces to 46 PE matmuls of [K=128, M=128 signals, N=512 cols] in
float32r (full-rate fp32 with 11-bit mantissa), accumulated in PSUM over the
2-3 k-blocks that cover each t-chunk's band.  The matmul orientation puts
signals on PSUM partitions and (t, c)-interleaved columns on the free axis,
so each PSUM tile is exactly a contiguous [128 signals, 128 t x 4 c] chunk of
the output layout — no transposes and fully contiguous 2 KiB-per-row output
DMAs.

Sharding: data-parallel over B (16 -> 2 per core); each core handles
2 x 512 = 1024 signals = 8 blocks of 128.
"""

from contextlib import ExitStack

import numpy as np

import concourse.bacc as bacc
import concourse.bass as bass
import concourse.tile as tile
from concourse import mybir
from concourse.bass_utils import run_bass_kernel_spmd

# ---------------------------------------------------------------- problem dims
import os  # noqa: E402

B, T, N = 16, 2048, 512
LEVELS = 3
C = LEVELS + 1
F = 8
NCORES = 8
B_PER_CORE = B // NCORES          # 2
TCHUNKS = T // 128                # 16
SCHUNKS = TCHUNKS + 1             # 17 sample chunks on the 64-shifted grid
SBLOCKS = B_PER_CORE * (N // 128)  # 8 signal blocks of 128 per core
OGROUP = int(os.environ.get("K_OGROUP", "8"))  # t-chunks per output DMA

DEC_LO = np.array([-0.010597401784997278, 0.032883011666982945, 0.030841381835986965,
                   -0.18703481171888114, -0.02798376941698385, 0.6308807679295904,
                   0.7148465705525415, 0.23037781330885523])
DEC_HI = np.array([-0.23037781330885523, 0.7148465705525415, -0.6308807679295904,
                   -0.02798376941698385, 0.18703481171888114, 0.030841381835986965,
                   -0.032883011666982945, -0.010597401784997278])


# ------------------------------------------------------- operator construction
def _dwt_step(sig, lo, hi):
    S = sig.shape[1]
    ext = np.pad(sig, ((0, 0), (F - 1, F - 1)), mode='symmetric')[:, 1:]
    L = (S + F - 1) // 2
    lo_r, hi_r = lo[::-1], hi[::-1]
    cA = sum(ext[:, k:k + 2 * L:2] * lo_r[k] for k in range(F))
    cD = sum(ext[:, k:k + 2 * L:2] * hi_r[k] for k in range(F))
    return cA, cD


def _resize(c, t):
    S = c.shape[-1]
    if S == t:
        return c
    if S > t:
        return c[..., :t]
    pos = (np.arange(t, dtype=c.dtype) + 0.5) * (S / t) - 0.5
    pos = np.clip(pos, 0.0, S - 1)
    lo = np.floor(pos).astype(np.int64)
    hi = np.minimum(lo + 1, S - 1)
    w = pos - lo.astype(c.dtype)
    return c[..., lo] * (1.0 - w) + c[..., hi] * w


def _build_operator():
    """M [T, T, C] float64: out[s, t, c] = sum_k sig[s, k] M[k, t, c]."""
    a = np.eye(T)
    details = []
    for _ in range(LEVELS):
        a, d = _dwt_step(a, DEC_LO, DEC_HI)
        details.append(d)
    coeffs = [a] + details[::-1]
    return np.stack([_resize(cf, T) for cf in coeffs], axis=-1)


def _plan():
    """Banded matmul schedule on the 64-shifted sample grid.

    The signal is staged in SBUF as 17 chunks of 128 samples covering
    [-64, 2112) (64 zero rows at each end).  Every t-chunk's operator band
    (width <= 194) then fits in exactly TWO consecutive shifted chunks
    [128j-64, 128j+192), vs 2-3 chunks (46 total) on the 0-aligned grid.
    Returns per-t-chunk block lists, the (tc, j) pair order, and the packed-M
    array [128, n_pairs, 512] whose partition p holds operator row 128j-64+p.
    """
    M = _build_operator()                      # [k, t, c]
    Mi = M.reshape(T, T * C)                   # col = 4 t + c
    Mpad = np.zeros((T + 128, T * C))          # row s+64 = operator row s
    Mpad[64:64 + T] = Mi
    kblocks, pairs = [], []
    for tc in range(TCHUNKS):
        cols = Mi[:, tc * 512:(tc + 1) * 512]
        rows = np.nonzero(np.any(cols != 0, axis=1))[0]
        jmin = (rows.min() + 64) // 128
        jmax = (rows.max() + 64) // 128
        blocks = list(range(jmin, jmax + 1))
        assert len(blocks) == 2, (tc, blocks)
        kblocks.append(blocks)
        for kb in blocks:
            pairs.append((tc, kb))
    packed = np.empty((128, len(pairs), 512), dtype=np.float32)
    for p, (tc, kb) in enumerate(pairs):
        packed[:, p, :] = Mpad[kb * 128:(kb + 1) * 128, tc * 512:(tc + 1) * 512]
    return kblocks, pairs, packed


_KBLOCKS, _PAIRS, _M_PACKED = _plan()
NPAIRS = len(_PAIRS)

F32 = mybir.dt.float32
F32R = mybir.dt.float32r
BF16 = mybir.dt.bfloat16

# M in bf16 halves the dominant input stream (11.75 -> 5.9 MiB per core).
# Walrus rejects mixed-dtype matmuls, so the signal is cast to bf16 in the
# load DMA (SWDGE casting DMA) when M is bf16.  float32r/float32r is the
# high-accuracy fallback (2.1e-4 vs ~1e-3 relative error).
_DT_CHOICE = os.environ.get("K_DTYPE", "bf16")
M_DTYPE = BF16 if _DT_CHOICE == "bf16" else F32R
SIG_DTYPE = M_DTYPE

if M_DTYPE == BF16:
    import ml_dtypes
    _M_PACKED = _M_PACKED.astype(ml_dtypes.bfloat16)

# Output band: fp16 halves the dominant store stream (32 -> 16 MiB per core)
# at a cost of ~5e-4 output-quantization relative error (fp16 keeps 11
# mantissa bits; |out| <= ~8 is far inside fp16 range).  Host casts back.
F16 = mybir.dt.float16
_ODT_CHOICE = os.environ.get("K_ODT", "f16")
OUT_DTYPE = {"f16": F16, "bf16": BF16, "f32": F32}[_ODT_CHOICE]
OUT_NP = {"f16": np.float16, "bf16": None, "f32": np.float32}[_ODT_CHOICE]


# ------------------------------------------------------------- device program
def _emit_body(tc_ctx, nc, x_d, o_d, pools, m_t, ogroup, probe=None,
               stage_marks=False):
    spool, opool, ppool = pools

    pair_idx = {pr: i for i, pr in enumerate(_PAIRS)}
    ncopy = 0
    for b in range(B_PER_CORE):
        for nb in range(N // 128):
            sb = b * (N // 128) + nb
            if stage_marks and sb in (2, 4, 6):
                # align the 4 staggered-reset stages to signal-block pairs so
                # stage preambles don't cut through a t-chunk's matmul/copy/
                # store chain
                tc_ctx.stage_boundary()
            sig = spool.tile([128, SCHUNKS, 128], SIG_DTYPE, name="sig")
            nc.sync.dma_start(sig[:], x_d[b, nb])
            for tci in range(TCHUNKS):
                blocks = _KBLOCKS[tci]
                acc = ppool.tile([128, 512], F32, name="acc")
                nmm = 1 if probe == "dma" else len(blocks)
                for j, kb in enumerate(blocks[:nmm]):
                    nc.tensor.matmul(
                        acc[:],
                        sig[:, kb, :],
                        m_t[:, pair_idx[(tci, kb)], :],
                        start=(j == 0), stop=(j == nmm - 1),
                    )
                if probe == "pe":
                    continue
                j = tci % ogroup
                if j == 0:
                    o_t = opool.tile([128, ogroup, 512], OUT_DTYPE, name="o_t")
                # PSUM -> SBUF (+ f32 -> OUT_DTYPE convert) round-robined over
                # DVE and Activation so neither becomes the serial stage
                if ncopy % 2 == 0:
                    nc.vector.tensor_copy(o_t[:, j, :], acc[:])
                else:
                    nc.scalar.copy(o_t[:, j, :], acc[:])
                ncopy += 1
                if probe == "nostore":
                    continue
                if j == ogroup - 1:
                    # batched store (OGROUP t-chunks -> one DMA) on the
                    # Activation HWDGE ring (SP keeps the input loads)
                    t0 = (tci - j) * 128
                    nc.scalar.dma_start(
                        o_d[b, nb * 128:(nb + 1) * 128, t0:t0 + ogroup * 128, :],
                        o_t[:],
                    )


SBUFS = int(os.environ.get("K_SBUFS", "3"))   # sig double-buffer depth
OBUFS = int(os.environ.get("K_OBUFS", "0"))   # 0 -> auto
# staggered_reset removes For_i's per-iteration all-engine barrier (sem
# resets move into rotating stage preambles), letting the tail of rep k
# overlap the head of rep k+1
STAGGERED = os.environ.get("K_STAGGERED", "1") == "1"
STAGEB = os.environ.get("K_STAGEB", "0") == "1"


def build_module(reps=1, ogroup=None, probe=None, stageb=None):
    """Build + compile the per-core Bass module.  reps>1 wraps the body in a
    hardware loop (used by test.py for wall-clock differencing timing)."""
    if ogroup is None:
        ogroup = OGROUP
    if stageb is None:
        stageb = STAGEB
    nc = bacc.Bacc("TRN2", target_bir_lowering=False, debug=False)
    x_d = nc.dram_tensor("x", [B_PER_CORE, N // 128, 128, SCHUNKS * 128],
                         SIG_DTYPE, kind="ExternalInput")
    m_d = nc.dram_tensor("m", [128, NPAIRS, 512], M_DTYPE, kind="ExternalInput")
    o_d = nc.dram_tensor("out", [B_PER_CORE, N, T, C], OUT_DTYPE,
                         kind="ExternalOutput")

    with tile.TileContext(nc) as tc_ctx, ExitStack() as ctx:
        mpool = ctx.enter_context(tc_ctx.tile_pool(name="mpool", bufs=1))
        pools = (
            ctx.enter_context(tc_ctx.tile_pool(name="spool", bufs=SBUFS)),
            ctx.enter_context(tc_ctx.tile_pool(name="opool",
                                             bufs=OBUFS or max(3, 24 // ogroup))),
            ctx.enter_context(tc_ctx.tile_pool(name="ppool", bufs=8, space="PSUM")),
        )
        # M is loop-invariant: one batched load before the rep loop; it stays
        # resident in SBUF, so steady-state iterations stream only x and out.
        m_t = mpool.tile([128, NPAIRS, 512], M_DTYPE, name="m_t")
        nc.sync.dma_start(m_t[:], m_d[:])
        if reps == 1:
            _emit_body(tc_ctx, nc, x_d, o_d, pools, m_t, ogroup, probe)
        else:
            with tc_ctx.For_i(0, reps, 1,
                              hint_engines=(mybir.EngineType.PE,
                                            mybir.EngineType.SP),
                              staggered_reset=STAGGERED):
                _emit_body(tc_ctx, nc, x_d, o_d, pools, m_t, ogroup, probe,
                           stage_marks=STAGGERED and stageb)

    nc.compile()
    return nc


_NC_CACHE = {}


def _get_module(reps=1, ogroup=None, probe=None, stageb=None):
    key = (reps, ogroup, probe, stageb)
    if key not in _NC_CACHE:
        _NC_CACHE[key] = build_module(reps, ogroup, probe, stageb)
    return _NC_CACHE[key]


# ------------------------------------------------------------------ entrypoint
def run(x, reps=1, ogroup=None, probe=None, stageb=None):
    """x: [16, 2048, 512, 1] float32 -> [16, 512, 2048, 4] float32."""
    nc = _get_module(reps, ogroup, probe, stageb)
    x3 = np.asarray(x)[:, :, :, 0]
    if SIG_DTYPE == BF16:
        import ml_dtypes
        x3 = x3.astype(ml_dtypes.bfloat16)
    else:
        x3 = x3.astype(np.float32)
    # pre-tile to the 64-shifted SBUF layout: sample s lives at
    # (chunk (s+64)//128, partition (s+64)%128); 64 zero rows pad each end
    xp = np.zeros((B, SCHUNKS * 128, N), dtype=x3.dtype)
    xp[:, 64:64 + T] = x3
    xt = np.ascontiguousarray(
        xp.reshape(B, SCHUNKS, 128, N // 128, 128).transpose(0, 3, 2, 1, 4)
        .reshape(B, N // 128, 128, SCHUNKS * 128))
    in_maps = [
        {"x": xt[c * B_PER_CORE:(c + 1) * B_PER_CORE], "m": _M_PACKED}
        for c in range(NCORES)
    ]
    res = run_bass_kernel_spmd(nc, in_maps, core_ids=list(range(NCORES)))
    out = np.concatenate([np.asarray(res.results[c]["out"])
                          for c in range(NCORES)], axis=0)
    return np.ascontiguousarray(out.astype(np.float32))


def kernel(x):
    return run(x)


# -------------------------------------------------------- timing-only runner
def make_timing_runner(reps, ogroup=None, probe=None, stageb=None):
    """Return a zero-copy executor for wall-clock differencing: inputs live
    on device across calls and outputs are never fetched to host (the caller
    only needs completion, via block_until_ready).  Mirrors the multi-core
    tail of bass2jax.run_bass_via_pjrt with the same jit/shard_map/donation
    setup; the measured NEFF is identical to run()'s."""
    import jax
    from jax.sharding import Mesh, PartitionSpec
    try:
        from jax.experimental.shard_map import shard_map
    except ImportError:
        from jax.shard_map import shard_map
    from concourse import bass2jax as B2J

    B2J.install_neuronx_cc_hook()
    nc = _get_module(reps, ogroup, probe, stageb)

    x = np.random.default_rng(0).standard_normal((B, T, N, 1)).astype(np.float32)
    x3 = x[:, :, :, 0]
    if SIG_DTYPE == BF16:
        import ml_dtypes
        x3 = x3.astype(ml_dtypes.bfloat16)
    xp = np.zeros((B, SCHUNKS * 128, N), dtype=x3.dtype)
    xp[:, 64:64 + T] = x3
    xt = np.ascontiguousarray(
        xp.reshape(B, SCHUNKS, 128, N // 128, 128).transpose(0, 3, 2, 1, 4)
        .reshape(B, N // 128, 128, SCHUNKS * 128))
    in_maps = [
        {"x": xt[c * B_PER_CORE:(c + 1) * B_PER_CORE], "m": _M_PACKED}
        for c in range(NCORES)
    ]

    partition_name = (nc.partition_id_tensor.name
                      if nc.partition_id_tensor else None)
    in_names, out_names, out_avals, zero_shapes = [], [], [], []
    import concourse.mybir as _mybir
    for alloc in nc.m.functions[0].allocations:
        if not isinstance(alloc, _mybir.MemoryLocationSet):
            continue
        name = alloc.memorylocations[0].name
        if alloc.kind == "ExternalInput":
            if name != partition_name:
                in_names.append(name)
        elif alloc.kind == "ExternalOutput":
            shape = tuple(alloc.tensor_shape)
            dtype = _mybir.dt.np(alloc.dtype)
            out_names.append(name)
            out_avals.append(jax.core.ShapedArray(shape, dtype))
            zero_shapes.append((shape, dtype))
    n_params = len(in_names)
    n_outs = len(out_avals)
    all_in_names = in_names + out_names + (
        [partition_name] if partition_name else [])
    donate = tuple(range(n_params, n_params + n_outs))

    def _body(*args):
        operands = list(args)
        if partition_name is not None:
            operands.append(B2J.partition_id_tensor())
        return tuple(B2J._bass_exec_p.bind(
            *operands,
            out_avals=tuple(out_avals),
            in_names=tuple(all_in_names),
            out_names=tuple(out_names),
            lowering_input_output_aliases=(),
            sim_require_finite=True,
            sim_require_nnan=True,
            nc=nc,
        ))

    devices = jax.devices()[:NCORES]
    mesh = Mesh(np.asarray(devices), ("core",))
    in_specs = (PartitionSpec("core"),) * (n_params + n_outs)
    out_specs = (PartitionSpec("core"),) * n_outs
    sharded = jax.jit(
        shard_map(_body, mesh=mesh, in_specs=in_specs, out_specs=out_specs,
                  check_rep=False),
        donate_argnums=donate, keep_unused=True,
    )
    sharding = jax.sharding.NamedSharding(mesh, PartitionSpec("core"))
    concat_in = [
        jax.device_put(
            np.concatenate([np.asarray(in_maps[c][nm]) for c in range(NCORES)],
                           axis=0), sharding)
        for nm in in_names
    ]

    # The kernel writes every element of every output each rep, so the
    # donated "zero" buffers never need re-zeroing: recycle the previous
    # call's outputs as the next call's donated outputs.  After warm-up a
    # timing call moves no bytes between host and device.
    state = [
        jax.device_put(np.zeros((NCORES * s[0], *s[1:]), dt), sharding)
        for s, dt in zero_shapes
    ]

    def _call():
        nonlocal state
        out = sharded(*concat_in, *state)
        jax.block_until_ready(out)
        state = list(out)

    _call()   # warm-up: jit trace + NEFF load
    return _call

